# revision 1
# baseline (speedup 1.0000x reference)
"""DistVisionTransformer (STOSA-style ViT, mean+cov paths) on 8 Trainium2
NeuronCores. Data-parallel: one image per core; full forward pass on-device
in float32r (TF32-like) matmuls with fp32 accumulation.

Layout: activations are feature-major [768, 394] where columns 0:197 are the
mean-path tokens and 197:394 the cov-path tokens (cls token at cols 0 / 197).
LayerNorm / softmax partition-axis reductions are done with ones-vector
matmuls on the PE; per-token broadcasts with rank-1 ones outer products.
"""
import numpy as np
from contextlib import ExitStack

import concourse.bass as bass
import concourse.bacc as bacc
import concourse.tile as tile
import concourse.mybir as mybir
from concourse.bass_utils import run_bass_kernel_spmd
from concourse.masks import make_identity

F32 = mybir.dt.float32
F32R = mybir.dt.float32r
AF = mybir.ActivationFunctionType
ALU = mybir.AluOpType

B, E, H, L, P, IMG, NCLS = 8, 768, 12, 12, 16, 224, 1000
D = E // H                  # 64
MLP = 4 * E                 # 3072
SCALE = D ** -0.5
NPATCH = (IMG // P) ** 2    # 196
N = NPATCH + 1              # 197
T2 = 2 * N                  # 394  (mean | cov concatenated along tokens)
NP256 = 256                 # padded token free-dim for 256-wide matmuls
KT = E // 128               # 6 k-tiles over features
MT_H = MLP // 128           # 24 hidden tiles
COVW = T2 - NP256           # 138: start of the 256-wide cov window
COVO = N - COVW             # 59: offset of cov data inside that window

_CACHE = {}


# --------------------------------------------------------------------------
# device kernel builder
# --------------------------------------------------------------------------

def build_nc(debug=False, repeats=1):
    nc = bacc.Bacc(None, target_bir_lowering=False)
    lp = nc.allow_low_precision("tf32-style kernel; fp32 psum accumulation")
    lp.__enter__()

    dp = nc.declare_dram_parameter
    xcol_d = dp("xcol", [E, NP256], F32R, isOutput=False)        # per-core im2col
    qkvw_d = dp("qkvw", [L, E, 3 * E], F32R, isOutput=False)     # [in, out] (q|k|v)
    projw_d = dp("projw", [L, E, E], F32R, isOutput=False)
    cprojw_d = dp("cprojw", [L, E, E], F32R, isOutput=False)
    fc1w_d = dp("fc1w", [L, E, MLP], F32R, isOutput=False)
    fc2w_d = dp("fc2w", [L, MLP, E], F32R, isOutput=False)
    pw_d = dp("pw", [E, E], F32R, isOutput=False)                # patch embed [in, out]
    cpw_d = dp("cpw", [E, E], F32R, isOutput=False)
    headw_d = dp("headw", [E, 1024], F32R, isOutput=False)       # [in, out] padded
    rpbT_d = dp("rpbT", [H, N, NP256], F32, isOutput=False)      # rpb[h].T, padded
    acols_d = dp("acols", [L, 128, 36], F32, isOutput=False)     # per-tile param cols
    bq_d = dp("bq", [L, 128, 24], F32, isOutput=False)           # qkv psum bias cols
    fc1b_d = dp("fc1b", [L, 128, 24], F32, isOutput=False)
    vb_d = dp("vb", [L, 1, 2 * E], F32R, isOutput=False)         # v / cov_v bias rows
    pbrow_d = dp("pbrow", [L, 1, 3 * E], F32R, isOutput=False)   # proj|cproj|fc2 bias rows
    ones_d = dp("ones", [128, 520], F32R, isOutput=False)        # all-ones block
    maskneg_d = dp("maskneg", [E, H], F32R, isOutput=False)      # -1 blockdiag
    clspb_d = dp("clspb", [E, 4], F32R, isOutput=False)          # cls|cov_cls|patch_b|cov_patch_b
    fcn_d = dp("fcn", [E, 2], F32, isOutput=False)               # fc_norm g|b
    headb_d = dp("headb", [128, 8], F32, isOutput=False)
    zeros_d = dp("zeros", [1, H * N + 64], F32R, isOutput=False)
    out_d = dp("out", [1024, 1], F32, isOutput=True)
    if debug:
        dbg_d = dp("dbg", [L + 1, E, T2], F32R, isOutput=True)

    with tile.TileContext(nc) as tc, ExitStack() as ctx:
        pp = ctx.enter_context(tc.tile_pool(name="persist", bufs=1))
        wp = ctx.enter_context(tc.tile_pool(name="weights", bufs=14))
        rp = ctx.enter_context(tc.tile_pool(name="rows", bufs=1))
        bp = ctx.enter_context(tc.tile_pool(name="bigscratch", bufs=2))
        ap = ctx.enter_context(tc.tile_pool(name="attn", bufs=2))
        ps = ctx.enter_context(tc.tile_pool(name="psum", bufs=8, space="PSUM"))

        _bn = [0]

        def bank(shape):
            _bn[0] += 1
            return ps.tile(shape, F32, tag="bank", name=f"pb{_bn[0]}")

        _wn = [0]

        def wchunk():
            _wn[0] += 1
            return wp.tile([128, E], F32R, tag="wchunk", name=f"w{_wn[0]}")

        # ---- persistent constants ----
        ones = pp.tile([128, 520], F32R)       # columns / rows of ones
        nc.sync.dma_start(ones[:], ones_d[:])
        onescol = ones[:, 0:1]
        onesrow = ones[0:1, :]
        mask = [pp.tile([128, H], F32R, tag=f"mask{t}", name=f"mask{t}") for t in range(KT)]
        for t in range(KT):
            nc.sync.dma_start(mask[t][:], maskneg_d[128 * t:128 * (t + 1), :])
        clspb = [pp.tile([128, 4], F32R, tag=f"clspb{t}", name=f"clspb{t}") for t in range(KT)]
        for t in range(KT):
            nc.sync.dma_start(clspb[t][:], clspb_d[128 * t:128 * (t + 1), :])
        ident = pp.tile([128, 128], F32)
        make_identity(nc, ident[:])

        # ---- persistent state / per-layer reused buffers ----
        xs = [pp.tile([128, T2], F32R, tag=f"xs{t}", name=f"xs{t}") for t in range(KT)]
        xn = [pp.tile([128, T2], F32R, tag=f"xn{t}", name=f"xn{t}") for t in range(KT)]
        xsq = [pp.tile([128, T2], F32R, tag=f"xsq{t}", name=f"xsq{t}") for t in range(KT)]
        qkvs = [pp.tile([128, T2], F32R, tag=f"qkvs{t}", name=f"qkvs{t}") for t in range(2 * KT)]
        sqk = [pp.tile([128, NP256], F32R, tag=f"sqk{t}", name=f"sqk{t}") for t in range(2 * KT)]
        vtm = {}
        for path in (0, 1):
            vtm[path] = (pp.tile([128, E], F32R, tag=f"vtm{path}a", name=f"vtm{path}a"),
                         pp.tile([N - 128, E], F32R, tag=f"vtm{path}b", name=f"vtm{path}b"))
        ctx_s = [pp.tile([128, T2], F32R, tag=f"ctx{t}", name=f"ctx{t}") for t in range(KT)]
        gath = pp.tile([1, H * N + 64], F32R)
        nc.sync.dma_start(gath[:], zeros_d[:])   # zero the pad columns once
        an_t = pp.tile([H, N], F32R)
        bn_t = pp.tile([H, N], F32)
        bT = (pp.tile([128, H], F32, tag="bT0", name="bT0"), pp.tile([N - 128, H], F32, tag="bT1", name="bT1"))
        stage = pp.tile([64, T2], F32R)
        eps_t = pp.tile([1, 1], F32)
        nc.vector.memset(eps_t[:], 1e-5)

        MTOK = (128, N - 128)      # token m-tile sizes
        eps = 1e-5

        def layer_norm(src, g_ap, b_ap):
            """src: 6 [128,T2] f32r tiles -> xn (feature-major LN over partitions)."""
            for t in range(KT):
                nc.scalar.activation(xsq[t][:], src[t][:], AF.Square)
            p_s = bank([1, T2])
            p_s2 = bank([1, T2])
            for t in range(KT):
                nc.tensor.matmul(p_s[:], onescol, src[t][:],
                                 start=(t == 0), stop=(t == KT - 1))
            for t in range(KT):
                nc.tensor.matmul(p_s2[:], onescol, xsq[t][:],
                                 start=(t == 0), stop=(t == KT - 1))
            mu = rp.tile([1, T2], F32, tag="ln_mu")
            nc.vector.tensor_scalar(mu[:], p_s[:], 1.0 / E, None, ALU.mult)
            var = rp.tile([1, T2], F32, tag="ln_var")
            nc.vector.tensor_scalar(var[:], p_s2[:], 1.0 / E, None, ALU.mult)
            musq = rp.tile([1, T2], F32, tag="ln_musq")
            nc.scalar.activation(musq[:], mu[:], AF.Square)
            nc.vector.tensor_sub(var[:], var[:], musq[:])
            std = rp.tile([1, T2], F32, tag="ln_std")
            nc.scalar.activation(std[:], var[:], AF.Sqrt, bias=eps_t[:])
            rinv = rp.tile([1, T2], F32, tag="ln_rinv")
            nc.vector.reciprocal(rinv[:], std[:])
            rinv_r = rp.tile([1, T2], F32R, tag="ln_rinvr")
            nc.scalar.activation(rinv_r[:], rinv[:], AF.Copy)
            mur = rp.tile([1, T2], F32R, tag="ln_mur")
            nc.vector.tensor_mul(mur[:], mu[:], rinv_r[:])
            p_R = bank([128, T2])
            p_MR = bank([128, T2])
            nc.tensor.matmul(p_R[:], onesrow[:, 0:128], rinv_r[:], start=True, stop=True)
            nc.tensor.matmul(p_MR[:], onesrow[:, 0:128], mur[:], start=True, stop=True)
            for t in range(KT):
                tmp = bp.tile([128, T2], F32, tag="ln_tmp")
                nc.vector.tensor_mul(tmp[:], src[t][:], p_R[:])
                nc.vector.tensor_sub(tmp[:], tmp[:], p_MR[:])
                nc.scalar.activation(xn[t][:], tmp[:], AF.Identity,
                                     bias=b_ap(t), scale=g_ap(t))

        for _rep in range(repeats):
            # ================= patch embed =================
            xcol = [pp.tile([128, NP256], F32R, tag=f"xcol{t}", name=f"xcol{t}") for t in range(KT)]
            for t in range(KT):
                nc.sync.dma_start(xcol[t][:], xcol_d[128 * t:128 * (t + 1), :])

            for path, w_d in ((0, pw_d), (1, cpw_d)):
                wt = []
                for kt in range(KT):
                    w = wchunk()
                    nc.sync.dma_start(w[:], w_d[128 * kt:128 * (kt + 1), :])
                    wt.append(w)
                for mt in range(KT):
                    pe = bank([128, NP256])
                    for kt in range(KT):
                        nc.tensor.matmul(pe[:], wt[kt][:, 128 * mt:128 * (mt + 1)],
                                         xcol[kt][:], start=(kt == 0), stop=(kt == KT - 1))
                    dst = xs[mt][:, 1 + path * N: 1 + path * N + NPATCH]
                    nc.scalar.activation(dst, pe[:, 0:NPATCH], AF.Identity,
                                         bias=clspb[mt][:, 2 + path: 3 + path], scale=1.0)
            # cls tokens into cols 0 / 197
            for t in range(KT):
                nc.vector.tensor_copy(
                    xs[t][:, 0:T2].rearrange("p (a c) -> p a c", a=2)[:, :, 0:1],
                    clspb[t][:, 0:2].rearrange("p (a c) -> p a c", a=2)[:, :, 0:1])

            if debug:
                for t in range(KT):
                    nc.sync.dma_start(dbg_d[0, 128 * t:128 * (t + 1), :], xs[t][:])

            # ================= transformer layers =================
            for li in range(L):
                A = rp.tile([128, 36], F32, tag="acols")
                nc.sync.dma_start(A[:], acols_d[li])
                Bq = rp.tile([128, 24], F32, tag="bqcols")
                nc.sync.dma_start(Bq[:], bq_d[li])
                f1b = rp.tile([128, 24], F32, tag="fc1b")
                nc.sync.dma_start(f1b[:], fc1b_d[li])
                vb = rp.tile([1, 2 * E], F32R, tag="vbrow")
                nc.sync.dma_start(vb[:], vb_d[li])
                pbr = rp.tile([1, 3 * E], F32R, tag="pbrow")
                nc.sync.dma_start(pbr[:], pbrow_d[li])

                def ac(t, c):
                    return A[:, 6 * t + c: 6 * t + c + 1]

                # ---- LN1 ----
                layer_norm(xs, lambda t: ac(t, 0), lambda t: ac(t, 1))

                # ---- qkv (q|k part), feature-major, both paths at once ----
                qkw = {}
                for b in (0, 1):
                    for kt in range(KT):
                        w = wchunk()
                        nc.sync.dma_start(w[:], qkvw_d[li, 128 * kt:128 * (kt + 1),
                                                       E * b:E * (b + 1)])
                        qkw[(b, kt)] = w
                for mt in range(2 * KT):
                    b, m = mt // KT, mt % KT
                    pq = bank([128, T2])
                    for kt in range(KT):
                        nc.tensor.matmul(pq[:], qkw[(b, kt)][:, 128 * m:128 * (m + 1)],
                                         xn[kt][:], start=(kt == 0), stop=(kt == KT - 1))
                    # mean columns
                    if mt < KT:   # q rows: * 2*SCALE, + 2*SCALE*q_bias
                        nc.vector.tensor_scalar(qkvs[mt][:, 0:N], pq[:, 0:N],
                                                2.0 * SCALE, Bq[:, 2 * mt:2 * mt + 1],
                                                ALU.mult, ALU.add)
                    else:
                        nc.vector.tensor_copy(qkvs[mt][:, 0:N], pq[:, 0:N])
                    # cov columns: elu(x + b) + 1 = exp(min(x+b,0)) + max(x+b,0)
                    e1 = bp.tile([128, N], F32, tag="elu1")
                    e2 = bp.tile([128, N], F32, tag="elu2")
                    bcol = Bq[:, 2 * mt + 1:2 * mt + 2]
                    nc.vector.tensor_scalar(e1[:], pq[:, N:T2], bcol, 0.0, ALU.add, ALU.min)
                    nc.scalar.activation(e1[:], e1[:], AF.Exp)
                    nc.vector.tensor_scalar(e2[:], pq[:, N:T2], bcol, 0.0, ALU.add, ALU.max)
                    nc.vector.tensor_add(qkvs[mt][:, N:T2], e1[:], e2[:])

                # sq = 2*sqrt(cq) = sqrt(4 cq); sk = sqrt(ck)
                for mt in range(2 * KT):
                    nc.scalar.activation(sqk[mt][:, 0:N], qkvs[mt][:, N:T2], AF.Sqrt,
                                         scale=4.0 if mt < KT else 1.0)

                # ---- v token-major (both paths), weight as moving operand ----
                for kt in range(KT):
                    w = wchunk()
                    nc.sync.dma_start(w[:], qkvw_d[li, 128 * kt:128 * (kt + 1), 2 * E:3 * E])
                    qkw[(2, kt)] = w
                for path in (0, 1):
                    for Mt in range(2):
                        moff, mw = 128 * Mt, MTOK[Mt]
                        for ch in range(2):
                            pv = bank([mw, 384])
                            for kt in range(KT):
                                nc.tensor.matmul(
                                    pv[:], xn[kt][:, path * N + moff: path * N + moff + mw],
                                    qkw[(2, kt)][:, 384 * ch:384 * (ch + 1)],
                                    start=(kt == 0), stop=False)
                            nc.tensor.matmul(pv[:], onesrow[:, 0:mw],
                                             vb[:, path * E + 384 * ch: path * E + 384 * (ch + 1)],
                                             start=False, stop=True)
                            dst = vtm[path][Mt][:, 384 * ch:384 * (ch + 1)]
                            if path == 0:
                                nc.vector.tensor_copy(dst, pv[:])
                            else:
                                e1 = bp.tile([128, 384], F32, tag="velu1")
                                e2 = bp.tile([128, 384], F32, tag="velu2")
                                nc.vector.tensor_scalar(e1[0:mw, :], pv[:], 0.0, None, ALU.min)
                                nc.scalar.activation(e1[0:mw, :], e1[0:mw, :], AF.Exp)
                                nc.vector.tensor_scalar(e2[0:mw, :], pv[:], 0.0, None, ALU.max)
                                nc.vector.tensor_add(dst, e1[0:mw, :], e2[0:mw, :])

                # ---- a/b norm terms via -1-blockdiag mask matmuls ----
                p_a = bank([H, T2])
                p_b = bank([H, T2])
                for half, pdst in ((0, p_a), (1, p_b)):
                    for kt in range(KT):
                        src = qkvs[half * KT + kt]
                        nc.scalar.activation(xsq[kt][:, 0:N], src[:, 0:N], AF.Square,
                                             scale=0.5 if half == 0 else 1.0)
                        nc.vector.tensor_copy(xsq[kt][:, N:T2], src[:, N:T2])
                    for kt in range(KT):
                        nc.tensor.matmul(pdst[:], mask[kt][:], xsq[kt][:],
                                         start=(kt == 0), stop=(kt == KT - 1))
                nc.vector.tensor_copy(an_t[:], p_a[:, 0:N])
                nc.vector.tensor_add(an_t[:], an_t[:], p_a[:, N:T2])
                nc.vector.tensor_copy(bn_t[:], p_b[:, 0:N])
                nc.vector.tensor_add(bn_t[:], bn_t[:], p_b[:, N:T2])
                # gather -a rows to partition 0 (strided per-head 256 slots)
                nc.sync.dma_start(
                    gath[:, 0:H * N].rearrange("p (h c) -> p h c", c=N), an_t[:])
                # transpose -b to per-token columns
                for Mt in range(2):
                    moff, mw = 128 * Mt, MTOK[Mt]
                    pt = bank([mw, H])
                    nc.tensor.transpose(pt[:], bn_t[:, moff:moff + mw], ident[0:H, 0:H])
                    nc.vector.tensor_copy(bT[Mt][:], pt[:])

                # ---- attention, head by head ----
                for h in range(H):
                    qt, off = h // 2, 64 * (h % 2)
                    rpb_h = (ap.tile([128, NP256], F32, tag="rpba", name="rpba"),
                             ap.tile([N - 128, NP256], F32, tag="rpbb", name="rpbb"))
                    nc.sync.dma_start(rpb_h[0][:], rpbT_d[h, 0:128, :])
                    nc.sync.dma_start(rpb_h[1][:], rpbT_d[h, 128:N, :])
                    Et, E2t, psc = [], [], []
                    for Mt in range(2):
                        moff, mw = 128 * Mt, MTOK[Mt]
                        pc = bank([mw, NP256])
                        nc.tensor.matmul(pc[:], qkvs[KT + qt][off:off + 64, moff:moff + mw],
                                         qkvs[qt][off:off + 64, 0:NP256],
                                         start=True, stop=False)
                        nc.tensor.matmul(pc[:], sqk[KT + qt][off:off + 64, moff:moff + mw],
                                         sqk[qt][off:off + 64, 0:NP256],
                                         start=False, stop=False)
                        nc.tensor.matmul(pc[:], onesrow[:, 0:mw],
                                         gath[:, N * h:N * h + NP256],
                                         start=False, stop=True)
                        psc.append(pc)
                    for Mt in range(2):
                        mw = MTOK[Mt]
                        sg = ap.tile([128, NP256], F32, tag="sig")
                        nc.scalar.activation(sg[0:mw, :], psc[Mt][:], AF.Sigmoid,
                                             bias=bT[Mt][:, h:h + 1])
                        nc.vector.tensor_add(sg[0:mw, :], sg[0:mw, :], rpb_h[Mt][:])
                        Ee = ap.tile([128, NP256], F32R, tag="E")
                        nc.scalar.activation(Ee[0:mw, :], sg[0:mw, :], AF.Exp)
                        Et.append(Ee)
                    pd = bank([1, NP256])
                    for Mt in range(2):
                        mw = MTOK[Mt]
                        nc.tensor.matmul(pd[:], onescol[0:mw, :], Et[Mt][0:mw, :],
                                         start=(Mt == 0), stop=(Mt == 1))
                    rd = ap.tile([1, NP256], F32, tag="rd", bufs=2)
                    nc.vector.reciprocal(rd[:], pd[:])
                    rdr = ap.tile([1, NP256], F32R, tag="rdr", bufs=2)
                    nc.scalar.activation(rdr[:], rd[:], AF.Copy)
                    pr = bank([128, NP256])
                    nc.tensor.matmul(pr[:], onesrow[:, 0:128], rdr[:], start=True, stop=True)
                    for Mt in range(2):
                        mw = MTOK[Mt]
                        nc.vector.tensor_mul(Et[Mt][0:mw, :], Et[Mt][0:mw, :], pr[0:mw, :])
                        E2 = ap.tile([128, NP256], F32R, tag="E2")
                        nc.vector.tensor_mul(E2[0:mw, :], Et[Mt][0:mw, :], Et[Mt][0:mw, :])
                        E2t.append(E2)
                    pctx = bank([64, 512])
                    for path in (0, 1):
                        src = Et if path == 0 else E2t
                        for Mt in range(2):
                            mw = MTOK[Mt]
                            nc.tensor.matmul(pctx[:, 256 * path:256 * (path + 1)],
                                             vtm[path][Mt][:, 64 * h:64 * (h + 1)],
                                             src[Mt][0:mw, :],
                                             start=(Mt == 0), stop=(Mt == 1))
                    pv = pctx[:, 0:512].rearrange("p (a c) -> p a c", a=2)[:, :, 0:N]
                    if off == 0:
                        nc.vector.tensor_copy(
                            ctx_s[qt][0:64, 0:T2].rearrange("p (a c) -> p a c", a=2), pv)
                    else:
                        nc.vector.tensor_copy(
                            stage[:, 0:T2].rearrange("p (a c) -> p a c", a=2), pv)
                        nc.sync.dma_start(ctx_s[qt][64:128, :], stage[:])

                # ---- proj / cov_proj + gamma1-scaled residual ----
                for path, w_d in ((0, projw_d), (1, cprojw_d)):
                    pwt = []
                    for kt in range(KT):
                        w = wchunk()
                        nc.sync.dma_start(w[:], w_d[li, 128 * kt:128 * (kt + 1), :])
                        pwt.append(w)
                    win = 0 if path == 0 else COVW
                    vo = 0 if path == 0 else COVO
                    for mt in range(KT):
                        pj = bank([128, NP256])
                        for kt in range(KT):
                            nc.tensor.matmul(pj[:], pwt[kt][:, 128 * mt:128 * (mt + 1)],
                                             ctx_s[kt][:, win:win + NP256],
                                             start=(kt == 0), stop=False)
                        nc.tensor.matmul(pj[:], pbr[:, path * E + 128 * mt:
                                                    path * E + 128 * (mt + 1)],
                                         onesrow[:, 0:NP256], start=False, stop=True)
                        nc.vector.scalar_tensor_tensor(
                            xs[mt][:, path * N:(path + 1) * N], pj[:, vo:vo + N],
                            ac(mt, 4), xs[mt][:, path * N:(path + 1) * N],
                            ALU.mult, ALU.add)

                # ---- LN2 + MLP (chunked fc2 accumulation) ----
                layer_norm(xs, lambda t: ac(t, 2), lambda t: ac(t, 3))
                pf2 = [ps.tile([128, T2], F32, tag="bank", name=f"pf2_{_i}") for _i in range(KT)]
                f1w = {}

                def load_f1(jb):
                    for kt in range(KT):
                        w = wchunk()
                        nc.sync.dma_start(w[:], fc1w_d[li, 128 * kt:128 * (kt + 1),
                                                       E * jb:E * (jb + 1)])
                        f1w[(jb, kt)] = w

                load_f1(0)
                for j in range(MT_H):
                    jb, jm = j // KT, j % KT
                    if jm == 0 and jb + 1 < 4:
                        load_f1(jb + 1)
                    ph = bank([128, T2])
                    for kt in range(KT):
                        nc.tensor.matmul(ph[:], f1w[(jb, kt)][:, 128 * jm:128 * (jm + 1)],
                                         xn[kt][:], start=(kt == 0), stop=(kt == KT - 1))
                    Hj = bp.tile([128, T2], F32R, tag="hj")
                    nc.scalar.activation(Hj[:], ph[:], AF.Gelu, bias=f1b[:, j:j + 1])
                    w2 = wchunk()
                    nc.sync.dma_start(w2[:], fc2w_d[li, 128 * j:128 * (j + 1), :])
                    for i in range(KT):
                        nc.tensor.matmul(pf2[i][:], w2[:, 128 * i:128 * (i + 1)], Hj[:],
                                         start=(j == 0), stop=False,
                                         skip_group_check=True)
                for i in range(KT):
                    nc.tensor.matmul(pf2[i][:], pbr[:, 2 * E + 128 * i:2 * E + 128 * (i + 1)],
                                     onesrow[:, 0:T2], start=False, stop=True,
                                     skip_group_check=True)
                    nc.vector.scalar_tensor_tensor(xs[i][:], pf2[i][:], ac(i, 5), xs[i][:],
                                                   ALU.mult, ALU.add)

                if debug:
                    for t in range(KT):
                        nc.sync.dma_start(dbg_d[li + 1, 128 * t:128 * (t + 1), :], xs[t][:])

            # ================= head =================
            pl = [rp.tile([128, 2], F32R, tag=f"pool{t}", name=f"pool{t}") for t in range(KT)]
            for t in range(KT):
                nc.vector.tensor_reduce(pl[t][:, 0:1], xs[t][:, 1:N], mybir.AxisListType.X,
                                        ALU.add)
                nc.vector.tensor_scalar(pl[t][:, 0:1], pl[t][:, 0:1], 1.0 / NPATCH,
                                        None, ALU.mult)
                nc.scalar.activation(pl[t][:, 1:2], pl[t][:, 0:1], AF.Square)
            p_s = bank([1, 2])
            for t in range(KT):
                nc.tensor.matmul(p_s[:], onescol, pl[t][:],
                                 start=(t == 0), stop=(t == KT - 1))
            mu = rp.tile([1, 2], F32, tag="hmu")
            nc.vector.tensor_scalar(mu[:], p_s[:], 1.0 / E, None, ALU.mult)
            musq = rp.tile([1, 1], F32, tag="hmusq")
            nc.scalar.activation(musq[:], mu[:, 0:1], AF.Square)
            var = rp.tile([1, 1], F32, tag="hvar")
            nc.vector.tensor_sub(var[:], mu[:, 1:2], musq[:])
            std = rp.tile([1, 1], F32, tag="hstd")
            nc.scalar.activation(std[:], var[:], AF.Sqrt, bias=eps_t[:])
            rinv = rp.tile([1, 1], F32, tag="hrinv")
            nc.vector.reciprocal(rinv[:], std[:])
            rr = rp.tile([1, 2], F32R, tag="hrr")
            nc.scalar.activation(rr[:, 0:1], rinv[:], AF.Copy)
            nc.vector.tensor_mul(rr[:, 1:2], mu[:, 0:1], rr[:, 0:1])
            p_bc = bank([128, 2])
            nc.tensor.matmul(p_bc[:], onesrow[:, 0:128], rr[:], start=True, stop=True)
            fcn = [rp.tile([128, 2], F32, tag=f"fcn{t}", name=f"fcn{t}") for t in range(KT)]
            tn = [rp.tile([128, 2], F32R, tag=f"tn{t}", name=f"tn{t}") for t in range(KT)]
            for t in range(KT):
                nc.sync.dma_start(fcn[t][:], fcn_d[128 * t:128 * (t + 1), :])
                tmp = rp.tile([128, 1], F32, tag="htmp")
                nc.vector.tensor_mul(tmp[:], pl[t][:, 0:1], p_bc[:, 0:1])
                nc.vector.tensor_sub(tmp[:], tmp[:], p_bc[:, 1:2])
                nc.scalar.activation(tn[t][:, 0:1], tmp[:], AF.Identity,
                                     bias=fcn[t][:, 1:2], scale=fcn[t][:, 0:1])
                nc.vector.tensor_copy(tn[t][:, 1:2], tn[t][:, 0:1])
            hwt = {}
            for blk in range(2):
                for kt in range(KT):
                    w = wchunk()
                    wd = 768 if blk == 0 else 256
                    nc.sync.dma_start(w[:, 0:wd],
                                      headw_d[128 * kt:128 * (kt + 1),
                                              768 * blk:768 * blk + wd])
                    hwt[(blk, kt)] = w
            hb = rp.tile([128, 8], F32, tag="headb")
            nc.sync.dma_start(hb[:], headb_d[:])
            osb = rp.tile([128, 8], F32, tag="osb")
            for mt in range(8):
                blk, mo = (0, mt) if mt < 6 else (1, mt - 6)
                po = bank([128, 2])
                for kt in range(KT):
                    nc.tensor.matmul(po[:], hwt[(blk, kt)][:, 128 * mo:128 * (mo + 1)],
                                     tn[kt][:], start=(kt == 0), stop=(kt == KT - 1))
                nc.scalar.activation(osb[:, mt:mt + 1], po[:, 0:1], AF.Identity,
                                     bias=hb[:, mt:mt + 1])
            nc.sync.dma_start(out_d[:].rearrange("(a p) c -> p a c", p=128),
                              osb[:].rearrange("p (a c) -> p a c", c=1))

    lp.__exit__(None, None, None)
    nc.finalize()
    return nc


# --------------------------------------------------------------------------
# host-side input preparation
# --------------------------------------------------------------------------

def prep_shared(i):
    """Build the shared (weights etc.) input map from the full input dict."""
    f = np.float32

    def g(k):
        return np.asarray(i[k], dtype=f)

    qkvw = np.ascontiguousarray(np.transpose(g("qkv_w"), (0, 2, 1)))
    projw = np.ascontiguousarray(np.transpose(g("proj_w"), (0, 2, 1)))
    cprojw = np.ascontiguousarray(np.transpose(g("cov_proj_w"), (0, 2, 1)))
    fc1w = np.ascontiguousarray(np.transpose(g("fc1_w"), (0, 2, 1)))
    fc2w = np.ascontiguousarray(np.transpose(g("fc2_w"), (0, 2, 1)))
    pw = np.ascontiguousarray(g("patch_w").reshape(E, E).T)
    cpw = np.ascontiguousarray(g("cov_patch_w").reshape(E, E).T)
    headw = np.zeros((E, 1024), f)
    headw[:, 0:NCLS] = g("head_w").T
    rpbT = np.zeros((H, N, NP256), f)
    rpbT[:, :, 0:N] = np.transpose(g("rel_pos_bias"), (0, 2, 1))

    acols = np.zeros((L, 128, 36), f)
    for c, k in enumerate(["norm1_g", "norm1_b", "norm2_g", "norm2_b",
                           "gamma1", "gamma2"]):
        v = g(k).reshape(L, KT, 128)
        for t in range(KT):
            acols[:, :, 6 * t + c] = v[:, t, :]
    bq = np.zeros((L, 128, 24), f)
    qb2 = (2.0 * SCALE) * g("q_bias")
    cqb = g("cov_q_bias")
    for mt in range(KT):
        bq[:, :, 2 * mt] = qb2[:, 128 * mt:128 * (mt + 1)]
        bq[:, :, 2 * mt + 1] = cqb[:, 128 * mt:128 * (mt + 1)]
    fc1b = np.ascontiguousarray(g("fc1_b").reshape(L, 24, 128).transpose(0, 2, 1))
    vb = np.concatenate([g("v_bias"), g("cov_v_bias")], axis=1)[:, None, :]
    pbrow = np.concatenate([g("proj_b"), g("cov_proj_b"), g("fc2_b")],
                           axis=1)[:, None, :]
    ones = np.ones((128, 520), f)
    maskneg = np.zeros((E, H), f)
    for h in range(H):
        maskneg[64 * h:64 * (h + 1), h] = -1.0
    clspb = np.zeros((E, 4), f)
    clspb[:, 0] = g("cls_tok").reshape(E)
    clspb[:, 1] = g("cov_cls_tok").reshape(E)
    clspb[:, 2] = g("patch_b")
    clspb[:, 3] = g("cov_patch_b")
    fcn = np.stack([g("fc_norm_g"), g("fc_norm_b")], axis=1)
    hbp = np.zeros(1024, f)
    hbp[0:NCLS] = g("head_b")
    headb = np.ascontiguousarray(hbp.reshape(8, 128).T)
    zeros = np.zeros((1, H * N + 64), f)
    return {
        "qkvw": qkvw, "projw": projw, "cprojw": cprojw, "fc1w": fc1w,
        "fc2w": fc2w, "pw": pw, "cpw": cpw, "headw": headw, "rpbT": rpbT,
        "acols": acols, "bq": bq, "fc1b": fc1b, "vb": vb, "pbrow": pbrow,
        "ones": ones, "maskneg": maskneg, "clspb": clspb, "fcn": fcn,
        "headb": headb, "zeros": zeros,
    }


def im2col(x):
    """x: [B,3,224,224] -> [B, 768, 256] (zero-padded cols)."""
    f = np.float32
    xc = np.asarray(x, dtype=f).reshape(B, 3, 14, 16, 14, 16)
    xc = xc.transpose(0, 1, 3, 5, 2, 4).reshape(B, E, NPATCH)
    out = np.zeros((B, E, NP256), f)
    out[:, :, 0:NPATCH] = xc
    return out


def _get_nc(debug=False, repeats=1):
    key = ("nc", debug, repeats)
    if key not in _CACHE:
        _CACHE[key] = build_nc(debug=debug, repeats=repeats)
    return _CACHE[key]


def run(inputs, debug=False, trace=False, repeats=1):
    nc = _get_nc(debug=debug, repeats=repeats)
    shared = prep_shared(inputs)
    xcols = im2col(inputs["x"])
    in_maps = [dict(shared, xcol=np.ascontiguousarray(xcols[b])) for b in range(B)]
    res = run_bass_kernel_spmd(nc, in_maps, list(range(B)), trace=trace)
    y = np.stack([res.results[b]["out"][0:NCLS, 0] for b in range(B)], axis=0)
    return y.astype(np.float32), res


def kernel(**inputs) -> np.ndarray:
    y, _ = run(inputs)
    return y



# revision 12
# speedup vs baseline: 1.4123x; 1.4123x over previous
"""DistVisionTransformer (STOSA-style ViT, mean+cov paths) on 8 Trainium2
NeuronCores. Data-parallel: one image per core; full forward pass on-device
in bf16 matmuls with fp32 PSUM accumulation.

Layout: activations are feature-major [768, 394] where columns 0:197 are the
mean-path tokens and 197:394 the cov-path tokens (cls token at cols 0 / 197).
LayerNorm / softmax partition-axis reductions are done with ones-vector
matmuls on the PE; per-token broadcasts with rank-1 ones outer products.

All scalar-engine transcendentals are expressed with {exp, ln, square,
identity, copy} (one activation table set) plus gelu, so each layer pays
exactly two ACT_TABLE_LOADs: sigmoid(x) = exp(-ln(1+exp(-x))) chains,
sqrt(c) = exp(0.5*ln(c)), 1/sqrt(v) = exp(-0.5*ln(v)), 1/d = exp(-ln(d)).
LayerNorm affine params, residual scales (gamma1/2) and fc_norm are folded
into adjacent weights on the host.
"""
import numpy as np
import ml_dtypes
from contextlib import ExitStack

import concourse.bass as bass
import concourse.bacc as bacc
import concourse.tile as tile
import concourse.mybir as mybir
from concourse.bass_utils import run_bass_kernel_spmd

F32 = mybir.dt.float32
BF16 = mybir.dt.bfloat16
AF = mybir.ActivationFunctionType
ALU = mybir.AluOpType

B, E, H, L, P, IMG, NCLS = 8, 768, 12, 12, 16, 224, 1000
D = E // H                  # 64
MLP = 4 * E                 # 3072
SCALE = D ** -0.5
NPATCH = (IMG // P) ** 2    # 196
N = NPATCH + 1              # 197
T2 = 2 * N                  # 394  (mean | cov concatenated along tokens)
NP256 = 256                 # padded token free-dim for 256-wide attn tiles
KT = E // 128               # 6 k-tiles over features
MT_H = MLP // 128           # 24 hidden tiles
COVW = T2 - NP256           # 138: start of the 256-wide cov window
COVO = N - COVW             # 59: offset of cov data inside that window
LN2C = 0.6931471805599453   # ln(2)

_CACHE = {}


class _Bacc(bacc.Bacc):
    """Bacc that remaps ln/exp activation-table loads onto the combined
    natural_log_exp_and_others set and drops the then-redundant loads.
    All our ACT functions are in {exp, ln, square, identity, copy} (all
    members of set 6) except gelu (set 10)."""

    def insert_act_table_loads(self):
        super().insert_act_table_loads()
        last = None
        for b in self.main_func.blocks:
            keep = []
            for i in b.instructions:
                if isinstance(i, mybir.InstLoadActFuncSet):
                    if i.act_func_set_id in (0, 5):
                        i.act_func_set_id = 6
                    if i.act_func_set_id == last:
                        continue
                    last = i.act_func_set_id
                keep.append(i)
            b.instructions[:] = keep


# --------------------------------------------------------------------------
# device kernel builder
# --------------------------------------------------------------------------

def build_nc(debug=False):
    nc = _Bacc(None, target_bir_lowering=False)
    for val in (1e-5, LN2C):
        t = nc.alloc_sbuf_tensor(f"const-f32-{val}", [128, 1], F32)
        nc.gpsimd.memset(t.ap(), val)
        nc.const_aps.aps[(F32, val)] = t.ap()
    nc.all_engine_barrier()
    lp = nc.allow_low_precision("bf16 kernel; fp32 psum accumulation")
    lp.__enter__()

    dp = nc.declare_dram_parameter
    xcol_d = dp("xcol", [E, NP256], BF16, isOutput=False)        # per-core im2col
    qkvw_d = dp("qkvw", [L, E, 3 * E], BF16, isOutput=False)     # [in, out] q|k|v
    projw_d = dp("projw", [L, E, 2 * E], BF16, isOutput=False)   # proj|cov_proj
    fc1w_d = dp("fc1w", [L, E, MLP], BF16, isOutput=False)
    fc2w_d = dp("fc2w", [L, MLP, E], BF16, isOutput=False)
    pw_d = dp("pw", [E, 2 * E], BF16, isOutput=False)            # patch|cov_patch
    headw_d = dp("headw", [E, 1024], BF16, isOutput=False)       # fc_norm folded
    rpbT_d = dp("rpbT", [H, N, NP256], BF16, isOutput=False)     # rpb[h].T padded
    acols_d = dp("acols", [L, 128, 48], F32, isOutput=False)     # per-layer cols
    vb_d = dp("vb", [L, 1, 2 * E], BF16, isOutput=False)         # v/cov_v bias rows
    pbrow_d = dp("pbrow", [L, 1, 3 * E], BF16, isOutput=False)   # proj|cproj|fc2 b
    ones_d = dp("ones", [128, 520], BF16, isOutput=False)
    maskneg_d = dp("maskneg", [E, H], BF16, isOutput=False)      # -1 blockdiag
    sel12_d = dp("sel12", [128, 144], BF16, isOutput=False)      # col-h ones blocks
    selrow_d = dp("selrow", [12, 1536], BF16, isOutput=False)    # row-h ones blocks
    clspb_d = dp("clspb", [E, 4], F32, isOutput=False)           # cls|cov_cls|pb|cpb
    headb_d = dp("headb", [128, 8], F32, isOutput=False)
    zeros_d = dp("zeros", [1, H * N + 64], BF16, isOutput=False)
    out_d = dp("out", [1024, 1], F32, isOutput=True)
    if debug:
        dbg_d = dp("dbg", [L + 1, E, T2], BF16, isOutput=True)

    with tile.TileContext(nc) as tc, ExitStack() as ctx:
        pp = ctx.enter_context(tc.tile_pool(name="persist", bufs=1))
        qkvp = ctx.enter_context(tc.tile_pool(name="qkvw", bufs=6))
        prjp = ctx.enter_context(tc.tile_pool(name="projw", bufs=6))
        f2p = ctx.enter_context(tc.tile_pool(name="fc2w", bufs=10))
        f1p = ctx.enter_context(tc.tile_pool(name="fc1w", bufs=7))
        rp = ctx.enter_context(tc.tile_pool(name="rows", bufs=1))
        bp = ctx.enter_context(tc.tile_pool(name="bigscratch", bufs=3))
        ap = ctx.enter_context(tc.tile_pool(name="attn", bufs=3))
        sgp = ctx.enter_context(tc.tile_pool(name="sigw", bufs=2))
        ps = ctx.enter_context(tc.tile_pool(name="psum", bufs=8, space="PSUM"))

        _bn = [0]

        def bank(shape):
            _bn[0] += 1
            return ps.tile(shape, F32, tag="bank", name=f"pb{_bn[0]}")

        # ---- persistent constants ----
        ones = pp.tile([128, 520], BF16)
        nc.sync.dma_start(ones[:], ones_d[:])
        onescol = ones[:, 0:1]
        onesrow = ones[0:1, :]
        mask = [pp.tile([128, H], BF16, tag=f"mask{t}", name=f"mask{t}") for t in range(KT)]
        for t in range(KT):
            nc.sync.dma_start(mask[t][:], maskneg_d[128 * t:128 * (t + 1), :])
        sel12 = pp.tile([128, 144], BF16)
        nc.sync.dma_start(sel12[:], sel12_d[:])
        selrow = pp.tile([12, 1536], BF16)
        nc.sync.dma_start(selrow[:], selrow_d[:])
        clspb = [pp.tile([128, 4], F32, tag=f"clspb{t}", name=f"clspb{t}") for t in range(KT)]
        for t in range(KT):
            nc.sync.dma_start(clspb[t][:], clspb_d[128 * t:128 * (t + 1), :])

        MTOK = (128, N - 128)      # token m-tile sizes
        # rel-pos bias, loaded once: [tokens, head-major query cols]
        rpb_s = (pp.tile([128, H * NP256], BF16, tag="rpb0", name="rpb0"),
                 pp.tile([N - 128, H * NP256], BF16, tag="rpb1", name="rpb1"))
        for Mt in range(2):
            moff, mw = 128 * Mt, MTOK[Mt]
            nc.sync.dma_start(
                rpb_s[Mt][:].rearrange("p (h c) -> p h c", h=H),
                rpbT_d[:, moff:moff + mw, :].rearrange("h p c -> p h c"))

        # ---- persistent state / per-layer reused buffers ----
        xs = [pp.tile([128, T2], BF16, tag=f"xs{t}", name=f"xs{t}") for t in range(KT)]
        xn = [pp.tile([128, T2], BF16, tag=f"xn{t}", name=f"xn{t}") for t in range(KT)]
        xsq = [pp.tile([128, T2], BF16, tag=f"xsq{t}", name=f"xsq{t}") for t in range(2 * KT)]
        qm = [pp.tile([128, NP256], BF16, tag=f"qm{t}", name=f"qm{t}") for t in range(2 * KT)]
        sqk = [pp.tile([128, NP256], BF16, tag=f"sqk{t}", name=f"sqk{t}") for t in range(2 * KT)]
        for t in range(2 * KT):    # pad query cols must stay finite
            nc.vector.memset(qm[t][:, N:NP256], 0.0)
            nc.vector.memset(sqk[t][:, N:NP256], 0.0)
        vtm = {}
        for path in (0, 1):
            vtm[path] = (pp.tile([128, E], BF16, tag=f"vtm{path}a", name=f"vtm{path}a"),
                         pp.tile([N - 128, E], BF16, tag=f"vtm{path}b", name=f"vtm{path}b"))
        Ew = (pp.tile([128, H * NP256], BF16, tag="Ew0", name="Ew0"),
              pp.tile([N - 128, H * NP256], BF16, tag="Ew1", name="Ew1"))
        ctx_s = [pp.tile([128, T2], BF16, tag=f"ctx{t}", name=f"ctx{t}") for t in range(KT)]
        gath = pp.tile([1, H * N + 64], BF16)
        nc.sync.dma_start(gath[:], zeros_d[:])   # zero the pad columns once
        bng = pp.tile([1, H * N + 64], BF16)
        nc.sync.dma_start(bng[:], zeros_d[:])
        an_t = pp.tile([H, N], BF16)
        bn_t = pp.tile([H, N], BF16)
        rcat = pp.tile([12, 512], BF16)
        stage = pp.tile([64, T2], BF16)

        def layer_norm():
            """xs (6 bf16 [128,T2] tiles) -> xn, no affine (folded into W).
            Uses xsq[0:KT] as square scratch."""
            for t in range(KT):
                nc.vector.tensor_mul(xsq[t][:], xs[t][:], xs[t][:])
            p_s = bank([1, T2])
            p_s2 = bank([1, T2])
            for t in range(KT):
                nc.tensor.matmul(p_s[:], onescol, xs[t][:],
                                 start=(t == 0), stop=(t == KT - 1))
            for t in range(KT):
                nc.tensor.matmul(p_s2[:], onescol, xsq[t][:],
                                 start=(t == 0), stop=(t == KT - 1))
            mu = rp.tile([1, T2], F32, tag="ln_mu")
            nc.vector.tensor_scalar(mu[:], p_s[:], 1.0 / E, None, ALU.mult)
            musq = rp.tile([1, T2], F32, tag="ln_musq")
            nc.vector.tensor_mul(musq[:], mu[:], mu[:])
            var = rp.tile([1, T2], F32, tag="ln_var")
            nc.vector.scalar_tensor_tensor(var[:], p_s2[:], 1.0 / E, musq[:],
                                           ALU.mult, ALU.subtract)
            lnv = rp.tile([1, T2], F32, tag="ln_lnv")
            nc.scalar.activation(lnv[:], var[:], AF.Ln, bias=1e-5)
            rinv = rp.tile([1, T2], BF16, tag="ln_rinv")
            nc.scalar.activation(rinv[:], lnv[:], AF.Exp, scale=-0.5)
            mur = rp.tile([1, T2], BF16, tag="ln_mur")
            nc.vector.tensor_mul(mur[:], mu[:], rinv[:])
            p_R = bank([128, T2])
            p_MR = bank([128, T2])
            nc.tensor.matmul(p_R[:], onesrow[:, 0:128], rinv[:], start=True, stop=True)
            nc.tensor.matmul(p_MR[:], onesrow[:, 0:128], mur[:], start=True, stop=True)
            for t in range(KT):
                tmp = bp.tile([128, T2], F32, tag="ln_tmp")
                nc.vector.tensor_mul(tmp[:], xs[t][:], p_R[:])
                nc.vector.tensor_sub(xn[t][:], tmp[:], p_MR[:])

        # ================= patch embed =================
        xcol = [pp.tile([128, NP256], BF16, tag=f"xcol{t}", name=f"xcol{t}") for t in range(KT)]
        for t in range(KT):
            nc.sync.dma_start(xcol[t][:], xcol_d[128 * t:128 * (t + 1), :])

        pwt = []
        for kt in range(KT):
            w = prjp.tile([128, 2 * E], BF16, tag="pw", name=f"pw{kt}")
            nc.gpsimd.dma_start(w[:], pw_d[128 * kt:128 * (kt + 1), :])
            pwt.append(w)
        for path in (0, 1):
            for mt in range(KT):
                pe = bank([128, NP256])
                for kt in range(KT):
                    nc.tensor.matmul(pe[:], pwt[kt][:, E * path + 128 * mt:E * path + 128 * (mt + 1)],
                                     xcol[kt][:], start=(kt == 0), stop=(kt == KT - 1))
                dst = xs[mt][:, 1 + path * N: 1 + path * N + NPATCH]
                nc.scalar.activation(dst, pe[:, 0:NPATCH], AF.Identity,
                                     bias=clspb[mt][:, 2 + path: 3 + path], scale=1.0)
        # cls tokens into cols 0 / 197
        for t in range(KT):
            nc.vector.tensor_copy(
                xs[t][:, 0:T2].rearrange("p (a c) -> p a c", a=2)[:, :, 0:1],
                clspb[t][:, 0:2].rearrange("p (a c) -> p a c", a=2)[:, :, 0:1])

        if debug:
            for t in range(KT):
                nc.sync.dma_start(dbg_d[0, 128 * t:128 * (t + 1), :], xs[t][:])

        # ================= transformer layers =================
        for li in range(L):
            A = rp.tile([128, 48], F32, tag="acols")
            nc.sync.dma_start(A[:], acols_d[li])
            vb = rp.tile([1, 2 * E], BF16, tag="vbrow")
            nc.sync.dma_start(vb[:], vb_d[li])
            pbr = rp.tile([1, 3 * E], BF16, tag="pbrow")
            nc.sync.dma_start(pbr[:], pbrow_d[li])

            # qkv weight chunks (q|k|v columns together)
            qkw = []
            for kt in range(KT):
                w = qkvp.tile([128, 3 * E], BF16, tag="qkw", name=f"qkw{li}_{kt}")
                nc.gpsimd.dma_start(w[:], qkvw_d[li, 128 * kt:128 * (kt + 1), :])
                qkw.append(w)

            # ---- LN1 ----
            layer_norm()

            # ---- qkv (q|k), feature-major, both paths at once ----
            for mt in range(2 * KT):
                b, m = mt // KT, mt % KT
                pq = bank([128, T2])
                for kt in range(KT):
                    nc.tensor.matmul(pq[:], qkw[kt][:, E * b + 128 * m:E * b + 128 * (m + 1)],
                                     xn[kt][:], start=(kt == 0), stop=(kt == KT - 1))
                # mean columns: q rows scaled by 2*SCALE (+ bias col)
                if mt < KT:
                    nc.vector.tensor_scalar(qm[mt][:, 0:N], pq[:, 0:N],
                                            2.0 * SCALE, A[:, mt:mt + 1],
                                            ALU.mult, ALU.add)
                else:
                    nc.vector.tensor_scalar(qm[mt][:, 0:N], pq[:, 0:N],
                                            A[:, mt:mt + 1], None, ALU.add)
                # mean squares for the -|q|^2 / -|k|^2 terms
                nc.scalar.activation(xsq[mt][:, 0:N], qm[mt][:, 0:N], AF.Square,
                                     scale=0.5 if mt < KT else 1.0)
                # cov columns: elu(x + b) + 1 = exp(min(x+b,0)) + max(x+b,0)
                e1 = bp.tile([128, N], BF16, tag="elu1")
                e2 = bp.tile([128, N], BF16, tag="elu2")
                bcol = A[:, 12 + mt:13 + mt]
                nc.vector.tensor_scalar(e1[:], pq[:, N:T2], bcol, 0.0, ALU.add, ALU.min)
                nc.scalar.activation(e1[:], e1[:], AF.Exp)
                nc.vector.tensor_scalar(e2[:], pq[:, N:T2], bcol, 0.0, ALU.add, ALU.max)
                nc.vector.tensor_add(xsq[mt][:, N:T2], e1[:], e2[:])
                # sq = 2*sqrt(cq) = exp(0.5 ln c + ln2); sk = exp(0.5 ln c)
                lc = bp.tile([128, N], F32, tag="lncov")
                nc.scalar.activation(lc[:], xsq[mt][:, N:T2], AF.Ln)
                nc.scalar.activation(sqk[mt][:, 0:N], lc[:], AF.Exp, scale=0.5,
                                     bias=LN2C if mt < KT else 0.0)

            # ---- v token-major (both paths), weight as moving operand ----
            for path in (0, 1):
                for Mt in range(2):
                    moff, mw = 128 * Mt, MTOK[Mt]
                    for ch in range(2):
                        pv = bank([mw, 384])
                        for kt in range(KT):
                            nc.tensor.matmul(
                                pv[:], xn[kt][:, path * N + moff: path * N + moff + mw],
                                qkw[kt][:, 2 * E + 384 * ch:2 * E + 384 * (ch + 1)],
                                start=(kt == 0), stop=False)
                        nc.tensor.matmul(pv[:], onesrow[:, 0:mw],
                                         vb[:, path * E + 384 * ch: path * E + 384 * (ch + 1)],
                                         start=False, stop=True)
                        dst = vtm[path][Mt][:, 384 * ch:384 * (ch + 1)]
                        if path == 0:
                            nc.vector.tensor_copy(dst, pv[:])
                        else:
                            e1 = bp.tile([128, 384], BF16, tag="velu1")
                            e2 = bp.tile([128, 384], BF16, tag="velu2")
                            nc.vector.tensor_scalar(e1[0:mw, :], pv[:], 0.0, None, ALU.min)
                            nc.scalar.activation(e1[0:mw, :], e1[0:mw, :], AF.Exp)
                            nc.vector.tensor_scalar(e2[0:mw, :], pv[:], 0.0, None, ALU.max)
                            nc.vector.tensor_add(dst, e1[0:mw, :], e2[0:mw, :])

            # ---- a/b norm terms via -1-blockdiag mask matmuls ----
            p_a = bank([H, T2])
            p_b = bank([H, T2])
            for half, pdst in ((0, p_a), (1, p_b)):
                for kt in range(KT):
                    nc.tensor.matmul(pdst[:], mask[kt][:], xsq[half * KT + kt][:],
                                     start=(kt == 0), stop=(kt == KT - 1))
            nc.vector.tensor_copy(an_t[:], p_a[:, 0:N])
            nc.vector.tensor_add(an_t[:], an_t[:], p_a[:, N:T2])
            nc.vector.tensor_copy(bn_t[:], p_b[:, 0:N])
            nc.vector.tensor_add(bn_t[:], bn_t[:], p_b[:, N:T2])
            # gather -a / -b rows to partition 0 (strided per-head N slots)
            nc.sync.dma_start(
                gath[:, 0:H * N].rearrange("p (h c) -> p h c", c=N), an_t[:])
            nc.sync.dma_start(
                bng[:, 0:H * N].rearrange("p (h c) -> p h c", c=N), bn_t[:])

            # ---- attention scores, head-pair by head-pair ----
            for pr2 in range(H // 2):
                for Mt in range(2):
                    moff, mw = 128 * Mt, MTOK[Mt]
                    psc = bank([mw, 512])
                    for hh in range(2):
                        h = 2 * pr2 + hh
                        qt, off = h // 2, 64 * (h % 2)
                        sl = psc[:, 256 * hh:256 * (hh + 1)]
                        nc.tensor.matmul(sl, qm[KT + qt][off:off + 64, moff:moff + mw],
                                         qm[qt][off:off + 64, 0:NP256],
                                         start=True, stop=False)
                        nc.tensor.matmul(sl, sqk[KT + qt][off:off + 64, moff:moff + mw],
                                         sqk[qt][off:off + 64, 0:NP256],
                                         start=False, stop=False)
                        nc.tensor.matmul(sl, onesrow[:, 0:mw],
                                         gath[:, N * h:N * h + NP256],
                                         start=False, stop=False)
                        nc.tensor.matmul(sl, bng[:, N * h + moff:N * h + moff + mw],
                                         onesrow[:, 0:NP256],
                                         start=False, stop=True)
                    # v = exp(score) for both heads of the pair; score <= ~0
                    # so v is bounded (ln stays in its legal input range)
                    nc.scalar.activation(Ew[Mt][:, 512 * pr2:512 * (pr2 + 1)],
                                         psc[:], AF.Exp)

            # softmax pipeline, wide across all heads:
            # sigmoid = v/(1+v) = v * exp(-ln(1+v)); E = exp(rpb + sigmoid)
            for Mt in range(2):
                mw = MTOK[Mt]
                w_t = Ew[Mt][0:mw, :]
                sg = sgp.tile([mw, H * NP256], BF16, tag="sigw")
                nc.scalar.activation(sg[:], w_t, AF.Ln, bias=1.0)
                nc.scalar.activation(sg[:], sg[:], AF.Exp, scale=-1.0)
                nc.vector.tensor_mul(w_t, w_t, sg[:])               # sigmoid
                nc.vector.tensor_add(w_t, w_t, rpb_s[Mt][:])        # + rpb
                nc.scalar.activation(w_t, w_t, AF.Exp)              # exp(logit)

            # denominators for all heads -> [12, 256]
            pd = bank([12, NP256])
            first = True
            for h in range(H):
                for Mt in range(2):
                    mw = MTOK[Mt]
                    nc.tensor.matmul(pd[:], sel12[0:mw, 12 * h:12 * (h + 1)],
                                     Ew[Mt][0:mw, 256 * h:256 * (h + 1)],
                                     start=first, stop=(h == H - 1 and Mt == 1))
                    first = False
            lnd = rp.tile([12, NP256], F32, tag="lnd")
            nc.scalar.activation(lnd[:], pd[:], AF.Ln)
            nc.scalar.activation(rcat[:, 0:256], lnd[:], AF.Exp, scale=-1.0)
            nc.scalar.activation(rcat[:, 256:512], lnd[:], AF.Exp, scale=-2.0)

            # ---- context: (V @ E) * r, (V2 @ E^2) * r^2 per head ----
            for h in range(H):
                qt, off = h // 2, 64 * (h % 2)
                prh = bank([128, 512])
                nc.tensor.matmul(prh[:], selrow[:, 128 * h:128 * (h + 1)],
                                 rcat[:], start=True, stop=True)
                prs = ap.tile([128, 512], BF16, tag="prs")
                nc.vector.tensor_copy(prs[:], prh[:])
                pctx = bank([64, 512])
                e2hs = []
                for Mt in range(2):
                    mw = MTOK[Mt]
                    esl = Ew[Mt][0:mw, 256 * h:256 * (h + 1)]
                    e2h = ap.tile([mw, NP256], BF16, tag="e2h")
                    nc.vector.tensor_mul(e2h[:], esl, esl)
                    e2hs.append(e2h)
                for Mt in range(2):
                    mw = MTOK[Mt]
                    nc.tensor.matmul(pctx[:, 0:256],
                                     vtm[0][Mt][:, 64 * h:64 * (h + 1)],
                                     Ew[Mt][0:mw, 256 * h:256 * (h + 1)],
                                     start=(Mt == 0), stop=(Mt == 1))
                for Mt in range(2):
                    nc.tensor.matmul(pctx[:, 256:512],
                                     vtm[1][Mt][:, 64 * h:64 * (h + 1)],
                                     e2hs[Mt][:], start=(Mt == 0), stop=(Mt == 1))
                if off == 0:
                    for path in (0, 1):
                        nc.vector.tensor_mul(
                            ctx_s[qt][0:64, path * N:(path + 1) * N],
                            pctx[:, 256 * path:256 * path + N],
                            prs[0:64, 256 * path:256 * path + N])
                else:
                    for path in (0, 1):
                        nc.vector.tensor_mul(
                            stage[:, path * N:(path + 1) * N],
                            pctx[:, 256 * path:256 * path + N],
                            prs[0:64, 256 * path:256 * path + N])
                    nc.sync.dma_start(ctx_s[qt][64:128, :], stage[:])

            # ---- proj / cov_proj + residual (gamma1 folded) ----
            prw = []
            for kt in range(KT):
                w = prjp.tile([128, 2 * E], BF16, tag="pw", name=f"prw{li}_{kt}")
                nc.gpsimd.dma_start(w[:], projw_d[li, 128 * kt:128 * (kt + 1), :])
                prw.append(w)
            for path in (0, 1):
                win = 0 if path == 0 else COVW
                vo = 0 if path == 0 else COVO
                for mt in range(KT):
                    pj = bank([128, NP256])
                    for kt in range(KT):
                        nc.tensor.matmul(pj[:], prw[kt][:, E * path + 128 * mt:
                                                        E * path + 128 * (mt + 1)],
                                         ctx_s[kt][:, win:win + NP256],
                                         start=(kt == 0), stop=False)
                    nc.tensor.matmul(pj[:], pbr[:, path * E + 128 * mt:
                                                path * E + 128 * (mt + 1)],
                                     onesrow[:, 0:NP256], start=False, stop=True)
                    nc.vector.tensor_add(xs[mt][:, path * N:(path + 1) * N],
                                         xs[mt][:, path * N:(path + 1) * N],
                                         pj[:, vo:vo + N])

            # ---- LN2 + MLP (chunked fc2 accumulation) ----
            layer_norm()
            pf2 = [ps.tile([128, T2], F32, tag="bank", name=f"pf2_{li}_{_i}")
                   for _i in range(KT)]
            f1w = {}

            def load_f1(jb):
                for kt in range(KT):
                    w = f1p.tile([128, E], BF16, tag="f1w", name=f"f1_{li}_{jb}_{kt}")
                    nc.gpsimd.dma_start(w[:], fc1w_d[li, 128 * kt:128 * (kt + 1),
                                                     E * jb:E * (jb + 1)])
                    f1w[(jb, kt)] = w

            def load_f2(j):
                w = f2p.tile([128, E], BF16, tag="f2w", name=f"f2_{li}_{j}")
                nc.gpsimd.dma_start(w[:], fc2w_d[li, 128 * j:128 * (j + 1), :])
                return w

            load_f1(0)
            w2s = {0: load_f2(0), 1: load_f2(1)}
            for j in range(MT_H):
                jb, jm = j // KT, j % KT
                if jm == 0 and jb + 1 < 4:
                    load_f1(jb + 1)
                if j + 2 < MT_H:
                    w2s[j + 2] = load_f2(j + 2)
                ph = bank([128, T2])
                for kt in range(KT):
                    nc.tensor.matmul(ph[:], f1w[(jb, kt)][:, 128 * jm:128 * (jm + 1)],
                                     xn[kt][:], start=(kt == 0), stop=(kt == KT - 1))
                Hj = bp.tile([128, T2], BF16, tag="hj")
                nc.scalar.activation(Hj[:], ph[:], AF.Gelu, bias=A[:, 24 + j:25 + j])
                w2 = w2s.pop(j)
                for i in range(KT):
                    nc.tensor.matmul(pf2[i][:], w2[:, 128 * i:128 * (i + 1)], Hj[:],
                                     start=(j == 0), stop=False,
                                     skip_group_check=True)
            for i in range(KT):
                nc.tensor.matmul(pf2[i][:], pbr[:, 2 * E + 128 * i:2 * E + 128 * (i + 1)],
                                 onesrow[:, 0:T2], start=False, stop=True,
                                 skip_group_check=True)
                nc.vector.tensor_add(xs[i][:], xs[i][:], pf2[i][:])

            if debug:
                for t in range(KT):
                    nc.sync.dma_start(dbg_d[li + 1, 128 * t:128 * (t + 1), :], xs[t][:])

        # ================= head =================
        pl = [rp.tile([128, 2], BF16, tag=f"pool{t}", name=f"pool{t}") for t in range(KT)]
        plm = [rp.tile([128, 1], F32, tag=f"poolm{t}", name=f"poolm{t}") for t in range(KT)]
        for t in range(KT):
            nc.vector.tensor_reduce(plm[t][:], xs[t][:, 1:N], mybir.AxisListType.X,
                                    ALU.add)
            nc.vector.tensor_scalar(pl[t][:, 0:1], plm[t][:], 1.0 / NPATCH,
                                    None, ALU.mult)
            nc.scalar.activation(pl[t][:, 1:2], pl[t][:, 0:1], AF.Square)
        p_s = bank([1, 2])
        for t in range(KT):
            nc.tensor.matmul(p_s[:], onescol, pl[t][:],
                             start=(t == 0), stop=(t == KT - 1))
        mu = rp.tile([1, 2], F32, tag="hmu")
        nc.vector.tensor_scalar(mu[:], p_s[:], 1.0 / E, None, ALU.mult)
        musq = rp.tile([1, 1], F32, tag="hmusq")
        nc.vector.tensor_mul(musq[:], mu[:, 0:1], mu[:, 0:1])
        var = rp.tile([1, 1], F32, tag="hvar")
        nc.vector.tensor_sub(var[:], mu[:, 1:2], musq[:])
        lnv = rp.tile([1, 1], F32, tag="hlnv")
        nc.scalar.activation(lnv[:], var[:], AF.Ln, bias=1e-5)
        rr = rp.tile([1, 2], BF16, tag="hrr")
        nc.scalar.activation(rr[:, 0:1], lnv[:], AF.Exp, scale=-0.5)
        nc.vector.tensor_mul(rr[:, 1:2], mu[:, 0:1], rr[:, 0:1])
        p_bc = bank([128, 2])
        nc.tensor.matmul(p_bc[:], onesrow[:, 0:128], rr[:], start=True, stop=True)
        tn = [rp.tile([128, 2], BF16, tag=f"tn{t}", name=f"tn{t}") for t in range(KT)]
        for t in range(KT):
            tmp = rp.tile([128, 1], F32, tag="htmp")
            nc.vector.tensor_mul(tmp[:], pl[t][:, 0:1], p_bc[:, 0:1])
            nc.vector.tensor_sub(tn[t][:, 0:1], tmp[:], p_bc[:, 1:2])
            nc.vector.tensor_copy(tn[t][:, 1:2], tn[t][:, 0:1])
        hwt = {}
        for blk in range(2):
            for kt in range(KT):
                w = f2p.tile([128, E], BF16, tag="f2w", name=f"hw{blk}_{kt}")
                wd = 768 if blk == 0 else 256
                nc.sync.dma_start(w[:, 0:wd],
                                  headw_d[128 * kt:128 * (kt + 1),
                                          768 * blk:768 * blk + wd])
                hwt[(blk, kt)] = w
        hb = rp.tile([128, 8], F32, tag="headb")
        nc.sync.dma_start(hb[:], headb_d[:])
        osb = rp.tile([128, 8], F32, tag="osb")
        for mt in range(8):
            blk, mo = (0, mt) if mt < 6 else (1, mt - 6)
            po = bank([128, 2])
            for kt in range(KT):
                nc.tensor.matmul(po[:], hwt[(blk, kt)][:, 128 * mo:128 * (mo + 1)],
                                 tn[kt][:], start=(kt == 0), stop=(kt == KT - 1))
            nc.scalar.activation(osb[:, mt:mt + 1], po[:, 0:1], AF.Identity,
                                 bias=hb[:, mt:mt + 1])
        nc.sync.dma_start(out_d[:].rearrange("(a p) c -> p a c", p=128),
                          osb[:].rearrange("p (a c) -> p a c", c=1))

    lp.__exit__(None, None, None)
    nc.finalize()
    return nc


# --------------------------------------------------------------------------
# host-side input preparation
# --------------------------------------------------------------------------

def prep_shared(i):
    """Build the shared (weights etc.) input map from the full input dict.
    Folds LN affines, residual scales and fc_norm into adjacent weights."""
    f = np.float32
    bf = ml_dtypes.bfloat16

    def g(k):
        return np.asarray(i[k], dtype=f)

    n1g, n1b = g("norm1_g"), g("norm1_b")           # [L, E]
    n2g, n2b = g("norm2_g"), g("norm2_b")
    g1, g2 = g("gamma1"), g("gamma2")

    qkvw = g("qkv_w")                               # [L, 3E, E]
    wbeta = np.einsum("loe,le->lo", qkvw, n1b)      # [L, 3E]
    qkvw = qkvw * n1g[:, None, :]
    qkvw_T = np.ascontiguousarray(qkvw.transpose(0, 2, 1)).astype(bf)

    projw = g("proj_w") * g1[:, :, None]            # [L, E, E], gamma1 on out
    cprojw = g("cov_proj_w") * g1[:, :, None]
    projcat = np.concatenate([projw.transpose(0, 2, 1),
                              cprojw.transpose(0, 2, 1)], axis=2)
    projcat = np.ascontiguousarray(projcat).astype(bf)

    fc1w = g("fc1_w")                               # [L, MLP, E]
    wb2 = np.einsum("lme,le->lm", fc1w, n2b)        # [L, MLP]
    fc1w = fc1w * n2g[:, None, :]
    fc1w_T = np.ascontiguousarray(fc1w.transpose(0, 2, 1)).astype(bf)

    fc2w = g("fc2_w") * g2[:, :, None]              # [L, E, MLP], gamma2 on out
    fc2w_T = np.ascontiguousarray(fc2w.transpose(0, 2, 1)).astype(bf)

    pwcat = np.concatenate([g("patch_w").reshape(E, E).T,
                            g("cov_patch_w").reshape(E, E).T], axis=1)
    pwcat = np.ascontiguousarray(pwcat).astype(bf)

    fcg, fcb = g("fc_norm_g"), g("fc_norm_b")
    headw = np.zeros((E, 1024), f)
    headw[:, 0:NCLS] = (g("head_w") * fcg[None, :]).T
    headw = headw.astype(bf)
    headb_eff = g("head_b") + g("head_w") @ fcb
    hbp = np.zeros(1024, f)
    hbp[0:NCLS] = headb_eff
    headb = np.ascontiguousarray(hbp.reshape(8, 128).T)

    rpbT = np.zeros((H, N, NP256), f)
    rpbT[:, :, 0:N] = np.transpose(g("rel_pos_bias"), (0, 2, 1))
    rpbT = rpbT.astype(bf)

    # per-layer parameter columns: qk mean bias (12), qk cov bias (12), fc1b (24)
    acols = np.zeros((L, 128, 48), f)
    qb_mean = np.concatenate([2.0 * SCALE * (g("q_bias") + wbeta[:, 0:E]),
                              wbeta[:, E:2 * E]], axis=1)          # [L, 2E]
    qb_cov = np.concatenate([g("cov_q_bias") + wbeta[:, 0:E],
                             wbeta[:, E:2 * E]], axis=1)
    for mt in range(2 * KT):
        acols[:, :, mt] = qb_mean[:, 128 * mt:128 * (mt + 1)]
        acols[:, :, 12 + mt] = qb_cov[:, 128 * mt:128 * (mt + 1)]
    fc1b_eff = g("fc1_b") + wb2                                    # [L, MLP]
    acols[:, :, 24:48] = fc1b_eff.reshape(L, 24, 128).transpose(0, 2, 1)

    vrow = np.concatenate([g("v_bias") + wbeta[:, 2 * E:],
                           g("cov_v_bias") + wbeta[:, 2 * E:]], axis=1)
    vb = vrow[:, None, :].astype(bf)
    pbrow = np.concatenate([g1 * g("proj_b"), g1 * g("cov_proj_b"),
                            g2 * g("fc2_b")], axis=1)[:, None, :].astype(bf)

    ones = np.ones((128, 520), bf)
    maskneg = np.zeros((E, H), f)
    for h in range(H):
        maskneg[64 * h:64 * (h + 1), h] = -1.0
    sel12 = np.zeros((128, 144), f)
    for h in range(12):
        sel12[:, 12 * h + h] = 1.0
    selrow = np.zeros((12, 1536), f)
    for h in range(12):
        selrow[h, 128 * h:128 * (h + 1)] = 1.0
    clspb = np.zeros((E, 4), f)
    clspb[:, 0] = g("cls_tok").reshape(E)
    clspb[:, 1] = g("cov_cls_tok").reshape(E)
    clspb[:, 2] = g("patch_b")
    clspb[:, 3] = g("cov_patch_b")
    zeros = np.zeros((1, H * N + 64), bf)
    return {
        "qkvw": qkvw_T, "projw": projcat, "fc1w": fc1w_T, "fc2w": fc2w_T,
        "pw": pwcat, "headw": headw, "rpbT": rpbT, "acols": acols,
        "vb": vb, "pbrow": pbrow, "ones": ones,
        "maskneg": maskneg.astype(bf), "sel12": sel12.astype(bf),
        "selrow": selrow.astype(bf), "clspb": clspb, "headb": headb,
        "zeros": zeros,
    }


def im2col(x):
    """x: [B,3,224,224] -> [B, 768, 256] (zero-padded cols, bf16)."""
    bf = ml_dtypes.bfloat16
    xc = np.asarray(x, dtype=np.float32).reshape(B, 3, 14, 16, 14, 16)
    xc = xc.transpose(0, 1, 3, 5, 2, 4).reshape(B, E, NPATCH)
    out = np.zeros((B, E, NP256), bf)
    out[:, :, 0:NPATCH] = xc
    return out


def _get_nc(debug=False):
    key = ("nc", debug)
    if key not in _CACHE:
        _CACHE[key] = build_nc(debug=debug)
    return _CACHE[key]


def run(inputs, debug=False, trace=False):
    nc = _get_nc(debug=debug)
    shared = prep_shared(inputs)
    xcols = im2col(inputs["x"])
    in_maps = [dict(shared, xcol=np.ascontiguousarray(xcols[b])) for b in range(B)]
    res = run_bass_kernel_spmd(nc, in_maps, list(range(B)), trace=trace)
    y = np.stack([res.results[b]["out"][0:NCLS, 0] for b in range(B)], axis=0)
    return y.astype(np.float32), res


def kernel(**inputs) -> np.ndarray:
    y, _ = run(inputs)
    return y
